# revision 19
# baseline (speedup 1.0000x reference)
"""Differentiable Gaussian-splat tile compositor on 8 Trainium2 cores.

Strategy: 16x8-pixel tiles (= exactly 128 pixels = one SBUF partition
block). The 32 global tile-rows are load-balanced onto the 8 cores (4 rows
each, greedy binpack of per-row gaussian-pair counts into 16 two-row "half"
slots). Each core processes its 64 tiles PIXEL-major: partitions carry a
tile's 128 local pixels, the free dim carries the depth-ordered packed
(gaussian, tile) columns (one contiguous segment per tile; pairs pruned at
tile-min-q > 2*ln(128)).

Device math (G = packed columns, 128-aligned halves):
  q[pix, g]  = Basis[12,128]^T @ A[12, G]      (PE, f32r hi/lo split)
  e          = exp(-q/2)                        (ACT -> fp16; alpha = e, the
                                                 1/255 threshold and 0.99
                                                 clamp are dropped; measured
                                                 rel-L2 impact ~3e-3)
  om         = 1 - e                            (DVE tensor_scalar fp16 4x;
                                                 last chunk on ACT via
                                                 Copy(-e+1) to keep the DVE
                                                 scan chain unbroken)
  T_excl     = scan(om shifted by 1, mult, max, boundary-mask u8)
               -- tensor_tensor_scan computes the per-tile EXCLUSIVE
               cumulative product of (1-alpha): state=(om[j-1]*state) max
               bmask[j]; bmask=1 at segment starts resets state to 1 (any
               product of om's is <= 1).                         (DVE)
  w          = e * T_excl                       (Pool; last chunk on DVE)
  w^T        = per-128-col block: DMA-engine xbar transpose (SBUF->SBUF,
               off the compute engines) for early blocks; PE transpose +
               DVE copy for the last 2 blocks (low latency on the tail)
  img_half   = colors_block^T @ w^T             (PE fp16 matmuls accumulating
               into 2 half PSUM tiles [96,128]; colors block-diagonal by
               tile)
Input DMAs are split/ordered so every consumer is scheduled after its DMA's
queue slice ends -- the tile scheduler then elides the DMA-completion waits
(~1.7us each). Output fp16, host converts and reassembles via the row
assignment.
"""

import os
import numpy as np

_H = 256
_W = 256
_NCORES = 8
_TW = 16                     # tile width
_TH = 8                      # tile height
_NTX = _W // _TW             # 16 tiles across
_NTY = (_H // _NCORES) // _TH  # 4 tile rows per band
_NT = _NTX * _NTY            # 64 tiles per core
_HALF_T = _NT // 2           # 32 tiles per output half
_NPIX = _TW * _TH            # 128 pixels per tile
_QTH = float(2.0 * np.log(255.0))
_QTH_PRUNE = float(2.0 * np.log(128.0))
_PAD_Q = 100.0


def _f32r_hi(x):
    xi = np.ascontiguousarray(x, dtype=np.float32).view(np.int32)
    return (xi & np.int32(~0x1FFF)).view(np.float32)


def _reference_numpy(means_2d, covs_2d, depth_features, color_features, H, W):
    """Exact slow fallback (mirrors reference.py math)."""
    order = np.argsort(depth_features, kind="stable")
    m = means_2d[order].astype(np.float32)
    cv = covs_2d[order].astype(np.float32)
    cl = color_features[order].astype(np.float32)
    a, b, c = cv[:, 0], cv[:, 1], cv[:, 2]
    det = a * c - b * b
    ia, ib, ic = c / det, -b / det, a / det
    xs = np.arange(W, dtype=np.float32) + 0.5
    ys = np.arange(H, dtype=np.float32) + 0.5
    img = np.zeros((3, H, W), np.float32)
    T = np.ones((H, W), np.float32)
    for p in range(m.shape[0]):
        dx = xs[None, :] - m[p, 0]
        dy = ys[:, None] - m[p, 1]
        q = ia[p] * dx * dx + 2.0 * ib[p] * dx * dy + ic[p] * dy * dy
        alpha = np.minimum(np.float32(0.99), np.exp(np.float32(-0.5) * q))
        alpha = np.where(alpha < 1.0 / 255.0, np.float32(0.0), alpha)
        w = alpha * T
        img += cl[p][:, None, None] * w[None]
        T = T * (1.0 - alpha)
    return img


def _row_tiles(r, m, ia, ib, ic, rx, ry):
    """Depth-ordered gaussian lists for the 16 tiles of global tile-row r
    (rows of 8 px at y0 = 8r), pruned with the relaxed q threshold."""
    tiles = []
    y0 = r * _TH
    for tx in range(_NTX):
        x0 = tx * _TW
        cand = np.nonzero(
            (m[:, 0] + rx >= x0 + 0.5 - 1e-6)
            & (m[:, 0] - rx <= x0 + _TW - 0.5 + 1e-6)
            & (m[:, 1] + ry >= y0 + 0.5 - 1e-6)
            & (m[:, 1] - ry <= y0 + _TH - 0.5 + 1e-6)
        )[0]
        if cand.size:
            dx = (x0 + 0.5 + np.arange(_TW))[None, :] - m[cand, 0][:, None]
            dy = (y0 + 0.5 + np.arange(_TH))[None, :] - m[cand, 1][:, None]
            q = (
                ia[cand][:, None, None] * (dx * dx)[:, None, :]
                + 2.0 * ib[cand][:, None, None]
                * dx[:, None, :] * dy[:, :, None]
                + ic[cand][:, None, None] * (dy * dy)[:, :, None]
            )
            qmin = q.reshape(cand.size, -1).min(axis=1)
            cand = cand[qmin <= _QTH_PRUNE + 1e-3]
        tiles.append(cand)
    return tiles


def _basis():
    lx = np.arange(_TW, dtype=np.float32) - (_TW - 1) / 2.0
    ly = np.arange(_TH, dtype=np.float32) - (_TH - 1) / 2.0
    xl = np.tile(lx, _TH)              # pixel p = ly*_TW + lx
    yl = np.repeat(ly, _TW)
    B = np.stack(
        [xl * xl, xl * yl, yl * yl, xl, yl, np.ones(_NPIX, np.float32)], 0
    )
    return np.concatenate([B, B], axis=0).astype(np.float32)   # [12, 128]


def _build_core_data(halves, row_tiles, G1, G, NB, m, ia, ib, ic, cl):
    """halves: ((rA, rB), (rC, rD)) tile-rows for this core's two output
    halves. Tile t of half h: row = halves[h][t // 16], tx = t % 16."""
    A = np.zeros((12, G), np.float32)
    A[5, :] = _PAD_Q
    bm = np.zeros((128, G), np.uint8)
    colors = np.zeros((128, NB * 96), np.float16)
    for h, base in ((0, 0), (1, G1)):
        j = 0
        for t in range(_HALF_T):
            r = halves[h][t // _NTX]
            tx = t % _NTX
            idx = row_tiles[r][tx]
            if not len(idx):
                continue
            bm[:, base + j] = 1
            cx = tx * _TW + _TW / 2.0
            cy = r * _TH + _TH / 2.0
            for g in idx:
                col = base + j
                mxl = m[g, 0] - cx
                myl = m[g, 1] - cy
                gia, gib, gic = ia[g], ib[g], ic[g]
                coef = np.array(
                    [
                        gia,
                        2.0 * gib,
                        gic,
                        -2.0 * (gia * mxl + gib * myl),
                        -2.0 * (gib * mxl + gic * myl),
                        gia * mxl * mxl + 2.0 * gib * mxl * myl
                        + gic * myl * myl,
                    ]
                )
                hi = _f32r_hi(coef.astype(np.float32))
                lo = (coef - hi.astype(np.float64)).astype(np.float32)
                A[:6, col] = hi
                A[6:, col] = lo
                b, rr = divmod(col, 128)
                cc = b * 96 + 3 * t + (0 if h == 0 else 0)
                colors[rr, cc: cc + 3] = cl[g]
                j += 1
    return A, bm, colors


def _build_program(G, NB1):
    from contextlib import ExitStack

    import concourse.bacc as bacc
    import concourse.tile as tile
    from concourse import mybir

    F32 = mybir.dt.float32
    F32R = mybir.dt.float32r
    F16 = mybir.dt.float16
    U8 = mybir.dt.uint8
    AF = mybir.ActivationFunctionType
    OP = mybir.AluOpType

    NB = G // 128
    NCOLS = NB * 96

    nc = bacc.Bacc(trn_type="TRN2", target_bir_lowering=False, debug=False)
    t_AB = nc.dram_tensor("AB", [12, 128 + G], F32, kind="ExternalInput")
    t_bm = nc.dram_tensor("bm", [128, G], U8, kind="ExternalInput")
    t_col = nc.dram_tensor("colors", [128, NCOLS], F16, kind="ExternalInput")
    t_id = nc.dram_tensor("ident", [128, 128], F16, kind="ExternalInput")
    t_out = nc.dram_tensor("out", [128, 192], F16, kind="ExternalOutput")

    # pipeline chunks over A columns: [0,384), [384,896), ... so that the
    # AB DMA chunk boundaries land at 512-col multiples of the AB tensor
    # (basis rides with chunk 0). Each DMA gen-slice then ends BEFORE its
    # consumer's scheduled start, so the tile scheduler elides the DMA
    # completion waits (which would otherwise cost ~1.7us each).
    if G <= 1024:
        bnds = [0, 384, 768, G]
    else:
        bnds = [0]
        while bnds[-1] < G:
            bnds.append(min(bnds[-1] + (384 if len(bnds) == 1 else 512), G))
    bnds = sorted(set(b for b in bnds if b <= G))
    nchunks = len(bnds) - 1

    with ExitStack() as ctx:
        tc = ctx.enter_context(tile.TileContext(nc))
        const = ctx.enter_context(tc.tile_pool(name="const", bufs=1))
        sbo = ctx.enter_context(tc.tile_pool(name="sbo", bufs=2))
        psq = ctx.enter_context(tc.tile_pool(name="psq", bufs=2, space="PSUM"))
        pswt = ctx.enter_context(tc.tile_pool(name="pswt", bufs=1, space="PSUM"))
        psim = ctx.enter_context(tc.tile_pool(name="psim", bufs=1, space="PSUM"))
        psw = ctx.enter_context(tc.tile_pool(name="psw", bufs=1, space="PSUM"))

        AB = const.tile([12, 128 + G], F32)
        bm_sb = const.tile([128, G], U8)
        col_sb = const.tile([128, NCOLS], F16)
        id_sb = const.tile([128, 128], F16)
        e_all = const.tile([128, G], F16)
        om_buf = const.tile([128, G + 1], F16)
        T_all = const.tile([128, G], F16)
        w_all = const.tile([128, G], F16)
        wt_all = const.tile([128, G], F16)

        # AB chunks alternate SP / gpsimd queues; bmask(u8) on SP after the
        # AB chunks; ident+colors on gpsimd after its AB chunks.
        for k in range(nchunks):
            a0, a1 = (0 if k == 0 else 128 + bnds[k]), 128 + bnds[k + 1]
            eng = nc.sync if k % 2 == 0 else nc.gpsimd
            eng.dma_start(
                AB[:, a0:a1].bitcast(F32R), t_AB[:, a0:a1].bitcast(F32R)
            )
        nc.sync.dma_start(bm_sb[:].bitcast(F32), t_bm[:].bitcast(F32))
        nc.gpsimd.dma_start(id_sb[:].bitcast(F32), t_id[:].bitcast(F32))
        nc.gpsimd.dma_start(col_sb[:].bitcast(F32), t_col[:].bitcast(F32))

        basis = AB[:, :128]

        # warm the PE clock while input DMAs are in flight
        warm = const.tile([128, 16], F32)
        nc.vector.memset(warm[:], 0.0)
        warm_ps = psw.tile([128, 16], F32, tag="warm")
        for _ in range(14):
            nc.tensor.matmul(
                warm_ps[:16, :16], warm[:], warm[:, :16], start=True, stop=True
            )
        nc.vector.memset(om_buf[:, 0:1], 0.0)

        imgT = psim.tile([128, 192], F32, tag="imgT")

        FAST_BLOCKS = 2
        nfast = NB - FAST_BLOCKS

        for c in range(nchunks):
            c0, n = bnds[c], bnds[c + 1] - bnds[c]
            q = psq.tile([128, n], F32, tag="q")
            nc.tensor.matmul(
                q[:],
                basis.bitcast(F32R),
                AB[:, 128 + c0: 128 + c0 + n].bitcast(F32R),
                start=True,
                stop=True,
            )
            nc.scalar.activation(e_all[:, c0: c0 + n], q[:], AF.Exp, scale=-0.5)
            if c == nchunks - 1:
                # last chunk's om on the (now idle) ACT engine: om = 1 - e
                # via Copy(-1*e + 1); keeps the DVE free for the scan chain
                nc.scalar.activation(
                    om_buf[:, 1 + c0: 1 + c0 + n], e_all[:, c0: c0 + n],
                    AF.Copy, bias=1.0, scale=-1.0,
                )
            else:
                nc.vector.tensor_scalar(
                    om_buf[:, 1 + c0: 1 + c0 + n], e_all[:, c0: c0 + n],
                    -1.0, 1.0, OP.mult, OP.add,
                )
            nc.vector.tensor_tensor_scan(
                T_all[:, c0: c0 + n],
                om_buf[:, c0: c0 + n],
                bm_sb[:, c0: c0 + n],
                1.0 if c0 == 0 else T_all[:, c0 - 1: c0],
                OP.mult,
                OP.max,
            )
            # w = e * T: mid chunks on Pool; the LAST chunk on DVE (saves a
            # cross-engine hop on the critical tail)
            if c < nchunks - 1:
                nc.gpsimd.tensor_tensor(
                    w_all[:, c0: c0 + n], e_all[:, c0: c0 + n],
                    T_all[:, c0: c0 + n], OP.mult
                )
            else:
                nc.vector.tensor_tensor(
                    w_all[:, c0: c0 + n], e_all[:, c0: c0 + n],
                    T_all[:, c0: c0 + n], OP.mult
                )
            # xbar transposes for fully-covered early blocks
            b0 = (c0 + 127) // 128
            b1 = (c0 + n) // 128
            for b in range(b0, min(b1, nfast)):
                nc.sync.dma_start(
                    wt_all[:, b * 128: (b + 1) * 128],
                    w_all[:, b * 128: (b + 1) * 128],
                    transpose=True,
                )

        # trailing blocks: PE transpose + DVE copy (low latency)
        nf = FAST_BLOCKS * 128
        wt_ps = pswt.tile([128, nf], F16, tag="wt")
        for k in range(FAST_BLOCKS):
            b = nfast + k
            nc.tensor.matmul(
                wt_ps[:, k * 128: (k + 1) * 128],
                w_all[:, b * 128: (b + 1) * 128],
                id_sb[:],
                is_transpose=True,
            )
        nc.vector.tensor_copy(wt_all[:, nfast * 128:], wt_ps[:])

        # image accumulation, TRANSPOSED: imgT[pixel, 3*tile+ch] so the
        # matmul free dim is 96 (40ns/block instead of 53) and both halves
        # share one PSUM tile
        for b in range(NB):
            h = 0 if b < NB1 else 1
            nc.tensor.matmul(
                imgT[:, 96 * h: 96 * h + 96],
                wt_all[:, b * 128: (b + 1) * 128],
                col_sb[:, b * 96: (b + 1) * 96],
                start=(b == 0 or b == NB1),
                stop=(b == NB1 - 1 or b == NB - 1),
            )
            if b == NB1 - 1:
                osb0 = sbo.tile([128, 96], F16, tag="osb0")
                nc.vector.tensor_copy(osb0[:], imgT[:, 0:96])
                nc.sync.dma_start(t_out[:, 0:96], osb0[:])
            elif b == NB - 1:
                # gpsimd cannot read PSUM; the (idle) ACT engine copies and
                # issues the final DMA on its own queue (no cross-engine hop)
                osb1 = sbo.tile([128, 96], F16, tag="osb1")
                nc.scalar.activation(osb1[:], imgT[:, 96:192], AF.Copy)
                nc.scalar.dma_start(t_out[:, 96:192], osb1[:])

    nc.compile()
    return nc


def _build_all(means_2d, covs_2d, depth_features, color_features):
    """Host prep: returns (nc, in_maps, assign) for the 8 cores."""
    order = np.argsort(depth_features, kind="stable")
    m = means_2d[order].astype(np.float64)
    cvo = covs_2d[order].astype(np.float64)
    cl = color_features[order].astype(np.float32)
    a, b, c = cvo[:, 0], cvo[:, 1], cvo[:, 2]
    det = a * c - b * b
    ia, ib, ic = c / det, -b / det, a / det
    rx = np.sqrt(_QTH * a) + 1e-3
    ry = np.sqrt(_QTH * c) + 1e-3

    nrows = _H // _TH        # 32 global tile-rows
    row_tiles = [_row_tiles(r, m, ia, ib, ic, rx, ry) for r in range(nrows)]
    loads = [sum(len(t) for t in row_tiles[r]) for r in range(nrows)]

    # binpack the 32 tile-rows into 16 half-slots of 2 rows each,
    # minimizing the max slot load (each slot = one output half of a core)
    slots = [[0, []] for _ in range(2 * _NCORES)]
    for r in sorted(range(nrows), key=lambda r: -loads[r]):
        cand = [s for s in slots if len(s[1]) < 2]
        s = min(cand, key=lambda s: s[0])
        s[0] += loads[r]
        s[1].append(r)
    Gh = max(s[0] for s in slots)
    Gh = (Gh + 127) // 128 * 128
    G1 = G2 = Gh
    G = G1 + G2
    NB = G // 128
    NB1 = G1 // 128

    # pair slots into cores (sorted for determinism)
    slot_rows = sorted(sorted(s[1]) for s in slots)
    assign = [(tuple(slot_rows[2 * k]), tuple(slot_rows[2 * k + 1]))
              for k in range(_NCORES)]

    basis = _basis()
    ident = np.eye(128, dtype=np.float16)
    in_maps = []
    for core in range(_NCORES):
        A, bm, colors = _build_core_data(
            assign[core], row_tiles, G1, G, NB, m, ia, ib, ic, cl
        )
        in_maps.append(
            {
                "AB": np.ascontiguousarray(
                    np.concatenate([basis, A], axis=1)
                ),
                "bm": bm,
                "colors": colors,
                "ident": ident,
            }
        )

    nc = _build_program(G, NB1)
    return nc, in_maps, assign


def kernel(means_2d, covs_2d, depth_features, color_features, height, width):
    H, W = int(height), int(width)
    means_2d = np.asarray(means_2d, np.float32)
    covs_2d = np.asarray(covs_2d, np.float32)
    depth_features = np.asarray(depth_features, np.float32)
    color_features = np.asarray(color_features, np.float32)

    a, b, c = (
        covs_2d[:, 0].astype(np.float64),
        covs_2d[:, 1].astype(np.float64),
        covs_2d[:, 2].astype(np.float64),
    )
    det = a * c - b * b
    if H != _H or W != _W or np.any(det <= 0) or np.any(a <= 0) or np.any(c <= 0):
        return _reference_numpy(
            means_2d, covs_2d, depth_features, color_features, H, W
        )

    nc, in_maps, assign = _build_all(
        means_2d, covs_2d, depth_features, color_features
    )
    if os.environ.get("GS_KERNEL_SIM") == "1":
        from types import SimpleNamespace

        from concourse.bass_interp import CoreSim

        results = []
        for core in range(_NCORES):
            sim = CoreSim(nc)
            for k, v in in_maps[core].items():
                sim.tensor(k)[:] = v
            sim.simulate()
            results.append({"out": np.array(sim.tensor("out"))})
        res = SimpleNamespace(results=results)
    else:
        from concourse.bass_utils import run_bass_kernel_spmd

        res = run_bass_kernel_spmd(nc, in_maps, core_ids=list(range(_NCORES)))

    img = np.zeros((3, _H, _W), np.float32)
    for core in range(_NCORES):
        o = np.asarray(res.results[core]["out"], np.float32)  # [128, 192]
        for h in range(2):
            for t in range(_HALF_T):
                r = assign[core][h][t // _NTX]
                tx = t % _NTX
                blk = o[:, 96 * h + 3 * t: 96 * h + 3 * t + 3]
                blk = blk.T.reshape(3, _TH, _TW)
                img[:, r * _TH: (r + 1) * _TH,
                    tx * _TW: (tx + 1) * _TW] = blk
    return img


# revision 20
# speedup vs baseline: 1.0403x; 1.0403x over previous
"""Differentiable Gaussian-splat tile compositor on 8 Trainium2 cores.

Strategy: 16x8-pixel tiles (= exactly 128 pixels = one SBUF partition
block). The 32 global tile-rows are load-balanced onto the 8 cores (4 rows
each, greedy binpack of per-row gaussian-pair counts into 16 two-row "half"
slots). Each core processes its 64 tiles PIXEL-major: partitions carry a
tile's 128 local pixels, the free dim carries the depth-ordered packed
(gaussian, tile) columns (one contiguous segment per tile; pairs pruned at
tile-min-q > 2*ln(128)).

Device math (G = packed columns, 128-aligned halves):
  q[pix, g]  = Basis[12,128]^T @ A[12, G]      (PE, f32r hi/lo split)
  e          = exp(-q/2)                        (ACT -> fp16; alpha = e, the
                                                 1/255 threshold and 0.99
                                                 clamp are dropped; measured
                                                 rel-L2 impact ~3e-3)
  om         = 1 - e                            (DVE tensor_scalar fp16 4x;
                                                 last chunk on ACT via
                                                 Copy(-e+1) to keep the DVE
                                                 scan chain unbroken)
  T_excl     = scan(om shifted by 1, mult, max, boundary-mask u8)
               -- tensor_tensor_scan computes the per-tile EXCLUSIVE
               cumulative product of (1-alpha): state=(om[j-1]*state) max
               bmask[j]; bmask=1 at segment starts resets state to 1 (any
               product of om's is <= 1).                         (DVE)
  w          = e * T_excl                       (Pool; last chunk on DVE)
  w^T        = per-128-col block: DMA-engine xbar transpose (SBUF->SBUF,
               off the compute engines) for early blocks; PE transpose +
               DVE copy for the last 2 blocks (low latency on the tail)
  img_half   = colors_block^T @ w^T             (PE fp16 matmuls accumulating
               into 2 half PSUM tiles [96,128]; colors block-diagonal by
               tile)
Input DMAs are split/ordered so every consumer is scheduled after its DMA's
queue slice ends -- the tile scheduler then elides the DMA-completion waits
(~1.7us each). Output fp16, host converts and reassembles via the row
assignment.
"""

import os
import numpy as np

_H = 256
_W = 256
_NCORES = 8
_TW = 16                     # tile width
_TH = 8                      # tile height
_NTX = _W // _TW             # 16 tiles across
_NTY = (_H // _NCORES) // _TH  # 4 tile rows per band
_NT = _NTX * _NTY            # 64 tiles per core
_HALF_T = _NT // 2           # 32 tiles per output half
_NPIX = _TW * _TH            # 128 pixels per tile
_QTH = float(2.0 * np.log(255.0))
_QTH_PRUNE = float(2.0 * np.log(128.0))
_PAD_Q = 100.0


def _f32r_hi(x):
    xi = np.ascontiguousarray(x, dtype=np.float32).view(np.int32)
    return (xi & np.int32(~0x1FFF)).view(np.float32)


def _reference_numpy(means_2d, covs_2d, depth_features, color_features, H, W):
    """Exact slow fallback (mirrors reference.py math)."""
    order = np.argsort(depth_features, kind="stable")
    m = means_2d[order].astype(np.float32)
    cv = covs_2d[order].astype(np.float32)
    cl = color_features[order].astype(np.float32)
    a, b, c = cv[:, 0], cv[:, 1], cv[:, 2]
    det = a * c - b * b
    ia, ib, ic = c / det, -b / det, a / det
    xs = np.arange(W, dtype=np.float32) + 0.5
    ys = np.arange(H, dtype=np.float32) + 0.5
    img = np.zeros((3, H, W), np.float32)
    T = np.ones((H, W), np.float32)
    for p in range(m.shape[0]):
        dx = xs[None, :] - m[p, 0]
        dy = ys[:, None] - m[p, 1]
        q = ia[p] * dx * dx + 2.0 * ib[p] * dx * dy + ic[p] * dy * dy
        alpha = np.minimum(np.float32(0.99), np.exp(np.float32(-0.5) * q))
        alpha = np.where(alpha < 1.0 / 255.0, np.float32(0.0), alpha)
        w = alpha * T
        img += cl[p][:, None, None] * w[None]
        T = T * (1.0 - alpha)
    return img


def _row_tiles(r, m, ia, ib, ic, rx, ry):
    """Depth-ordered gaussian lists for the 16 tiles of global tile-row r
    (rows of 8 px at y0 = 8r), pruned with the relaxed q threshold."""
    tiles = []
    y0 = r * _TH
    for tx in range(_NTX):
        x0 = tx * _TW
        cand = np.nonzero(
            (m[:, 0] + rx >= x0 + 0.5 - 1e-6)
            & (m[:, 0] - rx <= x0 + _TW - 0.5 + 1e-6)
            & (m[:, 1] + ry >= y0 + 0.5 - 1e-6)
            & (m[:, 1] - ry <= y0 + _TH - 0.5 + 1e-6)
        )[0]
        if cand.size:
            dx = (x0 + 0.5 + np.arange(_TW))[None, :] - m[cand, 0][:, None]
            dy = (y0 + 0.5 + np.arange(_TH))[None, :] - m[cand, 1][:, None]
            q = (
                ia[cand][:, None, None] * (dx * dx)[:, None, :]
                + 2.0 * ib[cand][:, None, None]
                * dx[:, None, :] * dy[:, :, None]
                + ic[cand][:, None, None] * (dy * dy)[:, :, None]
            )
            qmin = q.reshape(cand.size, -1).min(axis=1)
            cand = cand[qmin <= _QTH_PRUNE + 1e-3]
        tiles.append(cand)
    return tiles


def _basis():
    lx = np.arange(_TW, dtype=np.float32) - (_TW - 1) / 2.0
    ly = np.arange(_TH, dtype=np.float32) - (_TH - 1) / 2.0
    xl = np.tile(lx, _TH)              # pixel p = ly*_TW + lx
    yl = np.repeat(ly, _TW)
    B = np.stack(
        [xl * xl, xl * yl, yl * yl, xl, yl, np.ones(_NPIX, np.float32)], 0
    )
    return np.concatenate([B, B], axis=0).astype(np.float32)   # [12, 128]


def _build_core_data(halves, row_tiles, G1, G, NB, m, ia, ib, ic, cl):
    """halves: ((rA, rB), (rC, rD)) tile-rows for this core's two output
    halves. Tile t of half h: row = halves[h][t // 16], tx = t % 16."""
    A = np.zeros((12, G), np.float32)
    A[5, :] = _PAD_Q
    bm = np.zeros((128, G), np.uint8)
    colors = np.zeros((128, NB * 96), np.float16)
    for h, base in ((0, 0), (1, G1)):
        j = 0
        for t in range(_HALF_T):
            r = halves[h][t // _NTX]
            tx = t % _NTX
            idx = row_tiles[r][tx]
            if not len(idx):
                continue
            bm[:, base + j] = 1
            cx = tx * _TW + _TW / 2.0
            cy = r * _TH + _TH / 2.0
            for g in idx:
                col = base + j
                mxl = m[g, 0] - cx
                myl = m[g, 1] - cy
                gia, gib, gic = ia[g], ib[g], ic[g]
                coef = np.array(
                    [
                        gia,
                        2.0 * gib,
                        gic,
                        -2.0 * (gia * mxl + gib * myl),
                        -2.0 * (gib * mxl + gic * myl),
                        gia * mxl * mxl + 2.0 * gib * mxl * myl
                        + gic * myl * myl,
                    ]
                )
                hi = _f32r_hi(coef.astype(np.float32))
                lo = (coef - hi.astype(np.float64)).astype(np.float32)
                A[:6, col] = hi
                A[6:, col] = lo
                b, rr = divmod(col, 128)
                cc = b * 96 + 3 * t + (0 if h == 0 else 0)
                colors[rr, cc: cc + 3] = cl[g]
                j += 1
    return A, bm, colors


def _build_program(G, NB1):
    from contextlib import ExitStack

    import concourse.bacc as bacc
    import concourse.tile as tile
    from concourse import mybir

    F32 = mybir.dt.float32
    F32R = mybir.dt.float32r
    F16 = mybir.dt.float16
    U8 = mybir.dt.uint8
    AF = mybir.ActivationFunctionType
    OP = mybir.AluOpType

    NB = G // 128
    NCOLS = NB * 96

    nc = bacc.Bacc(trn_type="TRN2", target_bir_lowering=False, debug=False)
    t_AB = nc.dram_tensor("AB", [12, 128 + G], F32, kind="ExternalInput")
    t_bm = nc.dram_tensor("bm", [128, G], U8, kind="ExternalInput")
    t_col = nc.dram_tensor("colors", [128, NCOLS], F16, kind="ExternalInput")
    t_id = nc.dram_tensor("ident", [128, 128], F16, kind="ExternalInput")
    t_out = nc.dram_tensor("out", [192, 128], F16, kind="ExternalOutput")

    # pipeline chunks over A columns: [0,384), [384,896), ... so that the
    # AB DMA chunk boundaries land at 512-col multiples of the AB tensor
    # (basis rides with chunk 0). Each DMA gen-slice then ends BEFORE its
    # consumer's scheduled start, so the tile scheduler elides the DMA
    # completion waits (which would otherwise cost ~1.7us each).
    if G <= 1024:
        bnds = [0, 384, 768, G]
    else:
        bnds = [0]
        while bnds[-1] < G:
            bnds.append(min(bnds[-1] + (384 if len(bnds) == 1 else 512), G))
    bnds = sorted(set(b for b in bnds if b <= G))
    nchunks = len(bnds) - 1

    with ExitStack() as ctx:
        tc = ctx.enter_context(tile.TileContext(nc))
        const = ctx.enter_context(tc.tile_pool(name="const", bufs=1))
        sbo = ctx.enter_context(tc.tile_pool(name="sbo", bufs=2))
        psq = ctx.enter_context(tc.tile_pool(name="psq", bufs=2, space="PSUM"))
        pswt = ctx.enter_context(tc.tile_pool(name="pswt", bufs=1, space="PSUM"))
        psim = ctx.enter_context(tc.tile_pool(name="psim", bufs=1, space="PSUM"))
        psw = ctx.enter_context(tc.tile_pool(name="psw", bufs=1, space="PSUM"))

        AB = const.tile([12, 128 + G], F32)
        bm_sb = const.tile([128, G], U8)
        col_sb = const.tile([128, NCOLS], F16)
        id_sb = const.tile([128, 128], F16)
        e_all = const.tile([128, G], F16)
        om_buf = const.tile([128, G + 1], F16)
        T_all = const.tile([128, G], F16)
        w_all = const.tile([128, G], F16)
        wt_all = const.tile([128, G], F16)

        # AB chunks alternate SP / gpsimd queues; bmask(u8) on SP after the
        # AB chunks; ident+colors on gpsimd after its AB chunks.
        for k in range(nchunks):
            a0, a1 = (0 if k == 0 else 128 + bnds[k]), 128 + bnds[k + 1]
            eng = nc.sync if k % 2 == 0 else nc.gpsimd
            eng.dma_start(
                AB[:, a0:a1].bitcast(F32R), t_AB[:, a0:a1].bitcast(F32R)
            )
        nc.sync.dma_start(bm_sb[:].bitcast(F32), t_bm[:].bitcast(F32))
        nc.gpsimd.dma_start(id_sb[:].bitcast(F32), t_id[:].bitcast(F32))
        nc.gpsimd.dma_start(col_sb[:].bitcast(F32), t_col[:].bitcast(F32))

        basis = AB[:, :128]

        # warm the PE clock while input DMAs are in flight
        warm = const.tile([128, 16], F32)
        nc.vector.memset(warm[:], 0.0)
        warm_ps = psw.tile([128, 16], F32, tag="warm")
        for _ in range(14):
            nc.tensor.matmul(
                warm_ps[:16, :16], warm[:], warm[:, :16], start=True, stop=True
            )
        nc.vector.memset(om_buf[:, 0:1], 0.0)

        img = [psim.tile([96, 128], F32, tag=f"img{h}", name=f"img{h}")
               for h in range(2)]

        FAST_BLOCKS = 2
        nfast = NB - FAST_BLOCKS

        for c in range(nchunks):
            c0, n = bnds[c], bnds[c + 1] - bnds[c]
            q = psq.tile([128, n], F32, tag="q")
            nc.tensor.matmul(
                q[:],
                basis.bitcast(F32R),
                AB[:, 128 + c0: 128 + c0 + n].bitcast(F32R),
                start=True,
                stop=True,
            )
            nc.scalar.activation(e_all[:, c0: c0 + n], q[:], AF.Exp, scale=-0.5)
            if c == nchunks - 1:
                # last chunk's om on the (now idle) ACT engine: om = 1 - e
                # via Copy(-1*e + 1); keeps the DVE free for the scan chain
                nc.scalar.activation(
                    om_buf[:, 1 + c0: 1 + c0 + n], e_all[:, c0: c0 + n],
                    AF.Copy, bias=1.0, scale=-1.0,
                )
            else:
                nc.vector.tensor_scalar(
                    om_buf[:, 1 + c0: 1 + c0 + n], e_all[:, c0: c0 + n],
                    -1.0, 1.0, OP.mult, OP.add,
                )
            nc.vector.tensor_tensor_scan(
                T_all[:, c0: c0 + n],
                om_buf[:, c0: c0 + n],
                bm_sb[:, c0: c0 + n],
                1.0 if c0 == 0 else T_all[:, c0 - 1: c0],
                OP.mult,
                OP.max,
            )
            # w = e * T: mid chunks on Pool; the LAST chunk on DVE (saves a
            # cross-engine hop on the critical tail)
            if c < nchunks - 1:
                nc.gpsimd.tensor_tensor(
                    w_all[:, c0: c0 + n], e_all[:, c0: c0 + n],
                    T_all[:, c0: c0 + n], OP.mult
                )
            else:
                nc.vector.tensor_tensor(
                    w_all[:, c0: c0 + n], e_all[:, c0: c0 + n],
                    T_all[:, c0: c0 + n], OP.mult
                )
            # xbar transposes for fully-covered early blocks
            b0 = (c0 + 127) // 128
            b1 = (c0 + n) // 128
            for b in range(b0, min(b1, nfast)):
                nc.sync.dma_start(
                    wt_all[:, b * 128: (b + 1) * 128],
                    w_all[:, b * 128: (b + 1) * 128],
                    transpose=True,
                )

        # trailing blocks: PE transpose + DVE copy (low latency)
        nf = FAST_BLOCKS * 128
        wt_ps = pswt.tile([128, nf], F16, tag="wt")
        for k in range(FAST_BLOCKS):
            b = nfast + k
            nc.tensor.matmul(
                wt_ps[:, k * 128: (k + 1) * 128],
                w_all[:, b * 128: (b + 1) * 128],
                id_sb[:],
                is_transpose=True,
            )
        nc.vector.tensor_copy(wt_all[:, nfast * 128:], wt_ps[:])

        # image accumulation: per-block fp16 matmuls into the two half PSUMs
        for b in range(NB):
            h = 0 if b < NB1 else 1
            nc.tensor.matmul(
                img[h][:],
                col_sb[:, b * 96: (b + 1) * 96],
                wt_all[:, b * 128: (b + 1) * 128],
                start=(b == 0 or b == NB1),
                stop=(b == NB1 - 1 or b == NB - 1),
            )
            if b == NB1 - 1:
                osb0 = sbo.tile([96, 128], F16, tag="osb0")
                nc.vector.tensor_copy(osb0[:], img[0][:])
                nc.sync.dma_start(t_out[0:96, :], osb0[:])
            elif b == NB - 1:
                # gpsimd cannot read PSUM; the (idle) ACT engine copies and
                # issues the final DMA on its own queue (no cross-engine hop)
                osb1 = sbo.tile([96, 128], F16, tag="osb1")
                nc.scalar.activation(osb1[:], img[1][:], AF.Copy)
                nc.scalar.dma_start(t_out[96:192, :], osb1[:])

    nc.compile()
    return nc


def _build_all(means_2d, covs_2d, depth_features, color_features):
    """Host prep: returns (nc, in_maps, assign) for the 8 cores."""
    order = np.argsort(depth_features, kind="stable")
    m = means_2d[order].astype(np.float64)
    cvo = covs_2d[order].astype(np.float64)
    cl = color_features[order].astype(np.float32)
    a, b, c = cvo[:, 0], cvo[:, 1], cvo[:, 2]
    det = a * c - b * b
    ia, ib, ic = c / det, -b / det, a / det
    rx = np.sqrt(_QTH * a) + 1e-3
    ry = np.sqrt(_QTH * c) + 1e-3

    nrows = _H // _TH        # 32 global tile-rows
    row_tiles = [_row_tiles(r, m, ia, ib, ic, rx, ry) for r in range(nrows)]
    loads = [sum(len(t) for t in row_tiles[r]) for r in range(nrows)]

    # binpack the 32 tile-rows into 16 half-slots of 2 rows each,
    # minimizing the max slot load (each slot = one output half of a core)
    slots = [[0, []] for _ in range(2 * _NCORES)]
    for r in sorted(range(nrows), key=lambda r: -loads[r]):
        cand = [s for s in slots if len(s[1]) < 2]
        s = min(cand, key=lambda s: s[0])
        s[0] += loads[r]
        s[1].append(r)
    Gh = max(s[0] for s in slots)
    Gh = (Gh + 127) // 128 * 128
    G1 = G2 = Gh
    G = G1 + G2
    NB = G // 128
    NB1 = G1 // 128

    # pair slots into cores (sorted for determinism)
    slot_rows = sorted(sorted(s[1]) for s in slots)
    assign = [(tuple(slot_rows[2 * k]), tuple(slot_rows[2 * k + 1]))
              for k in range(_NCORES)]

    basis = _basis()
    ident = np.eye(128, dtype=np.float16)
    in_maps = []
    for core in range(_NCORES):
        A, bm, colors = _build_core_data(
            assign[core], row_tiles, G1, G, NB, m, ia, ib, ic, cl
        )
        in_maps.append(
            {
                "AB": np.ascontiguousarray(
                    np.concatenate([basis, A], axis=1)
                ),
                "bm": bm,
                "colors": colors,
                "ident": ident,
            }
        )

    nc = _build_program(G, NB1)
    return nc, in_maps, assign


def kernel(means_2d, covs_2d, depth_features, color_features, height, width):
    H, W = int(height), int(width)
    means_2d = np.asarray(means_2d, np.float32)
    covs_2d = np.asarray(covs_2d, np.float32)
    depth_features = np.asarray(depth_features, np.float32)
    color_features = np.asarray(color_features, np.float32)

    a, b, c = (
        covs_2d[:, 0].astype(np.float64),
        covs_2d[:, 1].astype(np.float64),
        covs_2d[:, 2].astype(np.float64),
    )
    det = a * c - b * b
    if H != _H or W != _W or np.any(det <= 0) or np.any(a <= 0) or np.any(c <= 0):
        return _reference_numpy(
            means_2d, covs_2d, depth_features, color_features, H, W
        )

    nc, in_maps, assign = _build_all(
        means_2d, covs_2d, depth_features, color_features
    )
    if os.environ.get("GS_KERNEL_SIM") == "1":
        from types import SimpleNamespace

        from concourse.bass_interp import CoreSim

        results = []
        for core in range(_NCORES):
            sim = CoreSim(nc)
            for k, v in in_maps[core].items():
                sim.tensor(k)[:] = v
            sim.simulate()
            results.append({"out": np.array(sim.tensor("out"))})
        res = SimpleNamespace(results=results)
    else:
        from concourse.bass_utils import run_bass_kernel_spmd

        res = run_bass_kernel_spmd(nc, in_maps, core_ids=list(range(_NCORES)))

    img = np.zeros((3, _H, _W), np.float32)
    for core in range(_NCORES):
        o = np.asarray(res.results[core]["out"], np.float32)  # [192, 128]
        for h in range(2):
            for t in range(_HALF_T):
                r = assign[core][h][t // _NTX]
                tx = t % _NTX
                blk = o[96 * h + 3 * t: 96 * h + 3 * t + 3].reshape(
                    3, _TH, _TW
                )
                img[:, r * _TH: (r + 1) * _TH,
                    tx * _TW: (tx + 1) * _TW] = blk
    return img
